# revision 13
# baseline (speedup 1.0000x reference)
"""Bass/Trainium2 kernel for nn_GPREDecoder (GlobalPointer relation-extraction loss).

Strategy: data-parallel over batch (B=8 -> 8 cores, 1 example per core).
Per example the device computes:
  - fp8 DoubleRow projection  projT = (alpha*W_all) @ x_aug.T  (bias folded),
    output channels permuted so q_head/k_head/q_ent0/k_ent0/q_ent1 land at
    row 0 of the five 128-row psum m-tiles (direct SBUF views after a bf16
    evacuation cast); the 3 remaining groups are regrouped by SBUF->SBUF DMA.
  - RoPE on DVE: stream_shuffle pair-swap + sign-folded sin table
    (rot(q) = q*cos + swap(q)*sin'), no J matmul, all bf16 2x-mode ops.
  - per-head S x S logits in bf16 on PE, tril masks added by identity-matmul
    accumulation, exp(SCALE/alpha^2 * logit) on ACT with fused per-row
    accumulation into a [128, 14] sums tile (2048-wide spans).
  - bf16 q/k tensors DMA'd out for the host-side multilabel-CE corrections
    (gathers of the 64 ground-truth pairs, computed in float64).
"""

import ml_dtypes
import numpy as np
from contextlib import ExitStack

import concourse.bass as bass
import concourse.mybir as mybir
import concourse.tile as tile
from concourse import bacc
from concourse.bass_utils import run_bass_kernel_spmd

B, S, HID, LAB = 8, 1024, 1024, 64
HD = 68
SCALE = 1.0 / HD**0.5
INF = 1.0e12
NCORES = 8
ALPHA = 16.0                  # fp8 weight pre-scale; exp scale divides alpha^2
ACT_SCALE = SCALE / (ALPHA * ALPHA)
NEG_BIG = -1.0e9 * ALPHA * ALPHA  # additive mask units match scaled logits
KPAD = 1152                   # 4 full double-row k-tiles (256ch) + 1 half (128ch)
MSLOT = 640                   # 5 m-tiles x 128 permuted output-channel slots

# qkout group order (our choice):
G_QHEAD, G_KHEAD, G_QTAIL, G_KTAIL, G_QE0, G_KE0, G_QE1, G_KE1 = range(8)
# original row offset of each 68-row group in W_all = [W_ent; W_head; W_tail]
_ORIG = {G_QE0: 0, G_KE0: 68, G_QE1: 136, G_KE1: 204,
         G_QHEAD: 272, G_KHEAD: 340, G_QTAIL: 408, G_KTAIL: 476}
# row-0 groups of m-tiles 0..4 (direct views of the dense tiles)
_ROW0 = [G_QHEAD, G_KHEAD, G_QE0, G_KE0, G_QE1]
# spill groups: (src_tile, src_row, cnt, dst_row) pieces
_SPILL = {
    G_KE1: [(0, 68, 60, 0), (1, 68, 8, 60)],
    G_QTAIL: [(1, 76, 52, 0), (2, 68, 16, 52)],
    G_KTAIL: [(2, 84, 44, 0), (3, 68, 24, 44)],
}
# tril m-tile widths and ACT packs: lists of (m, local_col) per pack
_TRIL_W = [S - 128 * m for m in range(8)]
_TRIL_PACKS = [
    [(0, 0), (1, 1024)],            # span 1920
    [(2, 0), (3, 768), (4, 1408)],  # span 1920
    [(5, 0), (6, 384), (7, 640)],   # span 768
]
_TRIL_SPANS = [1920, 1920, 768]

# heads: (name, q_operand, k_operand, tril?) resolved at build time
# ACT instruction order: head x4 pairs, tail x4 pairs, ent0 x3, ent1 x3
N_ACC = 14


def _slot_map():
    """slot (0..639) -> original W_all row, or -1 for pad."""
    slot = np.full(MSLOT, -1, np.int64)
    for t, g in enumerate(_ROW0):
        slot[t * 128: t * 128 + 68] = np.arange(_ORIG[g], _ORIG[g] + 68)
    for g, pieces in _SPILL.items():
        for (t, r, cnt, d) in pieces:
            slot[t * 128 + r: t * 128 + r + cnt] = np.arange(
                _ORIG[g] + d, _ORIG[g] + d + cnt)
    return slot


def _chunks_in_bank(base, lo, w):
    """Split [lo, lo+w) cols (absolute base+lo in the psum tile) at the
    512-col bank grid. Returns [(off, n)] with off relative to lo."""
    out = []
    off = 0
    while off < w:
        a = base + lo + off
        n = min(512 - (a % 512), w - off)
        out.append((off, n))
        off += n
    return out


def _build_nc():
    f32 = mybir.dt.float32
    bf16 = mybir.dt.bfloat16
    f8 = mybir.dt.float8e4
    Exp = mybir.ActivationFunctionType.Exp
    DR = mybir.MatmulPerfMode.DoubleRow

    nc = bacc.Bacc("TRN2", target_bir_lowering=False)

    xT8a = nc.dram_tensor("xT8a", [128, 8192], f8, kind="ExternalInput")
    xT8b = nc.dram_tensor("xT8b", [64, 2048], f8, kind="ExternalInput")
    # m-major weights: per (partition, m-tile) run of 5kt x 2 x 128 = 1280B
    wtb8m = nc.dram_tensor("wtb8m", [128, 6400], f8, kind="ExternalInput")
    cosT = nc.dram_tensor("cosT", [HD, S], bf16, kind="ExternalInput")
    sinT = nc.dram_tensor("sinT", [HD, S], bf16, kind="ExternalInput")
    aux = nc.dram_tensor("aux", [128, 256], bf16, kind="ExternalInput")
    sums = nc.dram_tensor("sums", [128, N_ACC], f32, kind="ExternalOutput")
    qkout = nc.dram_tensor("qkout", [8, HD, S], bf16, kind="ExternalOutput")

    with tile.TileContext(nc) as tc, ExitStack() as ctx:
        singles = ctx.enter_context(tc.tile_pool(name="singles", bufs=1))
        scratch = ctx.enter_context(tc.tile_pool(name="scratch", bufs=2))

        xT_sb = singles.tile([128, 5, 2, S], f8, tag="xT_sb", name="xT_sb")
        wtb_sb = singles.tile([128, 5, 5, 2, 128], f8, tag="wtb_sb",
                              name="wtb_sb")  # [p, t, kt, i, m]
        cos_sb = singles.tile([HD, S], bf16, tag="cos_sb", name="cos_sb")
        sin_sb = singles.tile([HD, S], bf16, tag="sin_sb", name="sin_sb")
        aux_sb = singles.tile([128, 256], bf16, tag="aux_sb", name="aux_sb")
        dense = [singles.tile([128, S], bf16, tag=f"dense{t}", name=f"dense{t}")
                 for t in range(5)]
        qk_sp = {g: singles.tile([HD, S], bf16, tag=f"sp{g}", name=f"sp{g}")
                 for g in _SPILL}
        qk_rope = {g: singles.tile([HD, S], bf16, tag=f"rp{g}", name=f"rp{g}")
                   for g in (G_QE0, G_KE0, G_QE1, G_KE1)}
        sums_sb = singles.tile([128, N_ACC], f32, tag="sums_sb", name="sums_sb")
        dummy = singles.tile([1, 8], f32, tag="dummy", name="dummy")

        I_sb = aux_sb[:, 0:128]
        mask_sb = aux_sb[:, 128:256]

        ps = ctx.enter_context(tc.tile_pool(name="ps", bufs=1, space="PSUM"))
        big = ps.tile([128, 4096], f32, tag="big", name="big")

        # ---- input DMAs: balanced across both HW rings; consts on SWDGE ----
        nc.vector.memset(dummy[:], 0.0)

        wtb8m_r = wtb8m.rearrange("p (t kt i m) -> p t kt i m", t=5, kt=5, i=2)
        xT8a_r = xT8a.rearrange("p (o i f) -> p o i f", o=4, i=2)

        # sync ring: wtb_t0, xT kt0, kt1, kt4   (proj01 critical path)
        nc.sync.dma_start(out=wtb_sb[:, 0], in_=wtb8m_r[:, 0])
        nc.sync.dma_start(out=xT_sb[:, 0:1], in_=xT8a_r[:, 0:1])
        nc.sync.dma_start(out=xT_sb[:, 1:2], in_=xT8a_r[:, 1:2])
        nc.sync.dma_start(out=xT_sb[0:64, 4], in_=xT8b.rearrange(
            "p (i f) -> p i f", i=2)[:, :, :])
        # scalar ring: wtb_t1, xT kt23, then lower-priority weights
        nc.scalar.dma_start(out=wtb_sb[:, 1], in_=wtb8m_r[:, 1])
        nc.scalar.dma_start(out=xT_sb[:, 2:4], in_=xT8a_r[:, 2:4])
        nc.scalar.dma_start(out=wtb_sb[:, 2], in_=wtb8m_r[:, 2])
        nc.scalar.dma_start(out=wtb_sb[:, 3], in_=wtb8m_r[:, 3])
        nc.scalar.dma_start(out=wtb_sb[:, 4], in_=wtb8m_r[:, 4])
        nc.scalar.activation(dummy[:], dummy[:], Exp)  # exp table warm
        # constants on the (otherwise idle) gpsimd software DGE
        nc.gpsimd.dma_start(out=sin_sb[:, :], in_=sinT[:, :])
        nc.gpsimd.dma_start(out=cos_sb[:, :], in_=cosT[:, :])
        nc.gpsimd.dma_start(out=aux_sb[:, :], in_=aux[:, :])

        # ---- helpers ----
        def proj(tiles_units):
            """Project m-tiles into psum units, kt-major (DMA-paced)."""
            for kt in range(5):
                p_hi = 64 if kt == 4 else 128
                for (t, u) in tiles_units:
                    base = u * 1024
                    for c in (0, 512):
                        nc.tensor.matmul(
                            big[0:128, base + c: base + c + 512],
                            wtb_sb[0:p_hi, t, kt, :, :],
                            xT_sb[0:p_hi, kt, :, c:c + 512],
                            start=(kt == 0), stop=(kt == 4),
                            perf_mode=DR,
                        )

        def evac(t, u, eng):
            src = big[0:128, u * 1024:(u + 1) * 1024]
            if eng == "act":
                nc.scalar.copy(out=dense[t][:, :], in_=src)
            else:
                nc.vector.tensor_copy(out=dense[t][:, :], in_=src)

        def pair_round(q_ap, k_ap, r0, u0):
            """Two non-tril rounds r0, r0+1 into units u0, u0+1 (2048 span)."""
            for j in (0, 1):
                m = r0 + j
                base = (u0 + j) * 1024
                for c in (0, 512):
                    nc.tensor.matmul(
                        big[0:128, base + c: base + c + 512],
                        q_ap[:, m * 128:(m + 1) * 128],
                        k_ap[:, c:c + 512],
                        start=True, stop=True,
                    )

        def tril_pack(q_ap, k_ap, pack, u0):
            """One tril ACT pack into units u0,u0+1; mask + logits matmuls."""
            base = u0 * 1024
            for (m, lo) in pack:
                w = _TRIL_W[m]
                g0 = 128 * m
                # diag block: mask first (start), logits joins (stop)
                nc.tensor.matmul(
                    big[0:128, base + lo: base + lo + 128],
                    I_sb, mask_sb, start=True, stop=False)
                nc.tensor.matmul(
                    big[0:128, base + lo: base + lo + 128],
                    q_ap[:, g0:g0 + 128], k_ap[:, g0:g0 + 128],
                    start=False, stop=True)
                for (off, n) in _chunks_in_bank(base, lo + 128, w - 128):
                    nc.tensor.matmul(
                        big[0:128, base + lo + 128 + off:
                            base + lo + 128 + off + n],
                        q_ap[:, g0:g0 + 128],
                        k_ap[:, g0 + 128 + off: g0 + 128 + off + n],
                        start=True, stop=True)

        acc_i = [0]

        def act_span(u0, span):
            i = acc_i[0]
            acc_i[0] += 1
            ap = big[0:128, u0 * 1024: u0 * 1024 + span]
            nc.scalar.activation(ap, ap, Exp, scale=ACT_SCALE,
                                 accum_out=sums_sb[:, i:i + 1])

        def rope(g, src_ap):
            """qk_rope[g] = src*cos + pairswap(src)*sin' (DVE)."""
            sh = scratch.tile([HD, S], bf16, tag="sh", name=f"sh{g}")
            tmp = scratch.tile([HD, S], bf16, tag="tmp", name=f"tmp{g}")
            swap_mask = [i ^ 1 for i in range(32)]
            nc.vector.stream_shuffle(sh[:, :], src_ap, swap_mask)
            nc.vector.tensor_tensor(tmp[:, :], sh[:, :], sin_sb[:, :],
                                    mybir.AluOpType.mult)
            nc.vector.tensor_tensor(qk_rope[g][:, :], src_ap, cos_sb[:, :],
                                    mybir.AluOpType.mult)
            nc.vector.tensor_tensor(qk_rope[g][:, :], qk_rope[g][:, :],
                                    tmp[:, :], mybir.AluOpType.add)

        # ---- PE: projections 0-3 (01 first: they gate the ACT stream) ----
        proj([(0, 0), (1, 1)])
        proj([(2, 2), (3, 3)])

        # ---- evacuations: dense0 on DVE, dense1 on ACT, 2/3 on DVE ----
        evac(0, 0, "dve")
        evac(1, 1, "act")
        evac(2, 2, "dve")
        evac(3, 3, "dve")

        q_head = dense[0][0:HD, :]
        k_head = dense[1][0:HD, :]
        q_ent0_raw = dense[2][0:HD, :]
        k_ent0_raw = dense[3][0:HD, :]

        # ---- spills + early qkout on the sync HW ring (idle after inputs) --
        for g, pieces in _SPILL.items():
            for (t, r, cnt, d) in pieces:
                nc.sync.dma_start(out=qk_sp[g][d:d + cnt, :],
                                  in_=dense[t][r:r + cnt, :])
        nc.sync.dma_start(out=qkout[G_QHEAD], in_=q_head)
        nc.sync.dma_start(out=qkout[G_KHEAD], in_=k_head)
        for g in (G_QTAIL, G_KTAIL):
            nc.sync.dma_start(out=qkout[g], in_=qk_sp[g][:, :])

        # ---- DVE: rope for ent0 (ent1 after evac4) ----
        rope(G_QE0, q_ent0_raw)
        rope(G_KE0, k_ent0_raw)

        # ---- PE + ACT: head pairs A-D, proj4 squeezed into the U01 cycle ---
        pair_round(q_head, k_head, 0, 0)      # headA @U01
        act_span(0, 2048)
        pair_round(q_head, k_head, 2, 2)      # headB @U23
        act_span(2, 2048)
        pair_round(q_head, k_head, 4, 0)      # headC @U01
        act_span(0, 2048)
        pair_round(q_head, k_head, 6, 2)      # headD @U23
        act_span(2, 2048)

        proj([(4, 0)])                        # proj4 @U0 after headC drained
        evac(4, 0, "dve")
        rope(G_QE1, dense[4][0:HD, :])
        rope(G_KE1, qk_sp[G_KE1][:, :])
        for g in (G_QE0, G_KE0, G_QE1, G_KE1):
            nc.sync.dma_start(out=qkout[g], in_=qk_rope[g][:, :])

        q_tail = qk_sp[G_QTAIL][:, :]
        k_tail = qk_sp[G_KTAIL][:, :]
        pair_round(q_tail, k_tail, 0, 2)      # tailA @U23
        act_span(2, 2048)
        pair_round(q_tail, k_tail, 2, 0)      # tailB @U01 (after evac4)
        act_span(0, 2048)
        pair_round(q_tail, k_tail, 4, 2)      # tailC @U23
        act_span(2, 2048)
        pair_round(q_tail, k_tail, 6, 0)      # tailD @U01
        act_span(0, 2048)

        qe0, ke0 = qk_rope[G_QE0][:, :], qk_rope[G_KE0][:, :]
        qe1, ke1 = qk_rope[G_QE1][:, :], qk_rope[G_KE1][:, :]
        units = [2, 0, 2, 0, 2, 0]
        plan = [(qe0, ke0, 0), (qe0, ke0, 1), (qe0, ke0, 2),
                (qe1, ke1, 0), (qe1, ke1, 1), (qe1, ke1, 2)]
        for (qa, ka, pi), u0 in zip(plan, units):
            tril_pack(qa, ka, _TRIL_PACKS[pi], u0)
            act_span(u0, _TRIL_SPANS[pi])

        # sums out on the scalar queue: no cross-engine hop after last exp
        nc.scalar.dma_start(out=sums[:, :], in_=sums_sb[:, :])

    nc.finalize()
    return nc


_NC_CACHE = None


def _get_nc():
    global _NC_CACHE
    if _NC_CACHE is None:
        _NC_CACHE = _build_nc()
    return _NC_CACHE


def _host_tables():
    pos = np.arange(S, dtype=np.float64)[:, None]
    inv = np.power(10000.0, -2.0 * np.arange(HD // 2, dtype=np.float64) / HD)
    ang = pos * inv                                    # [S, 34]
    cosr = np.repeat(np.cos(ang), 2, axis=1).T         # [68, S]
    sinr = np.repeat(np.sin(ang), 2, axis=1).T
    # sign-folded sin: rot = x*cos + swap(x)*sin'; sin'[2i] = -sin, [2i+1] = +
    sgn = np.where(np.arange(HD) % 2 == 0, -1.0, 1.0)[:, None]
    cosT = cosr.astype(ml_dtypes.bfloat16)
    sinT = (sinr * sgn).astype(ml_dtypes.bfloat16)
    auxm = np.zeros((128, 256), np.float32)
    auxm[:, 0:128] = np.eye(128, dtype=np.float32)
    auxm[:, 128:256] = np.where(
        np.arange(128)[None, :] >= np.arange(128)[:, None], 0.0, NEG_BIG)
    return cosT, sinT, auxm.astype(ml_dtypes.bfloat16)


def _mcce_host(E_dev, q, k, gt):
    """pos/neg multilabel-CE for one (example, head). q,k: [68,S] f64; gt [P,2]."""
    i = gt[:, 0].astype(np.int64)
    j = gt[:, 1].astype(np.int64)
    flat = i * S + j
    lv = np.sum(q[:, i] * k[:, j], axis=0) * SCALE     # [P]
    live = flat != 0
    pos_loss = np.log1p(np.sum(np.exp(-lv[live])))
    l00 = float(np.sum(q[:, 0] * k[:, 0]) * SCALE)
    uf, ui = np.unique(flat, return_index=True)
    keep = uf != 0
    excl = np.exp(l00) + np.sum(np.exp(lv[ui[keep]]))
    neg_loss = np.log1p(E_dev - excl)
    return pos_loss + neg_loss


def _reference_numpy(hidden, entity_labels, attention_mask, gt_entity, gt_head,
                     gt_tail, ent_emb, W_ent, b_ent, W_head, b_head, W_tail,
                     b_tail):
    """Slow exact numpy fallback (used only if attention_mask is not all-ones)."""
    x = np.concatenate([hidden, ent_emb[entity_labels]], axis=-1)

    def rope_np(v):
        b, s, h, d = v.shape
        pos = np.arange(s, dtype=np.float32)[:, None]
        inv = np.power(10000.0, -2.0 * np.arange(d // 2, dtype=np.float32) / d)
        ang = pos * inv
        sin = np.repeat(np.sin(ang), 2, axis=-1)[None, :, None, :]
        cos = np.repeat(np.cos(ang), 2, axis=-1)[None, :, None, :]
        v2 = np.stack([-v[..., 1::2], v[..., ::2]], axis=-1).reshape(v.shape)
        return v * cos + v2 * sin

    def gp(x, W, b, mask, heads, use_rope, tril):
        bx, sx, _ = x.shape
        proj = (x @ W.T + b).reshape(bx, sx, heads, 2 * HD)
        qw, kw = proj[..., :HD], proj[..., HD:]
        if use_rope:
            qw, kw = rope_np(qw), rope_np(kw)
        logits = np.einsum('bmhd,bnhd->bhmn', qw, kw) * SCALE
        pad = mask[:, None, None, :]
        logits = logits * pad - (1.0 - pad) * INF
        if tril:
            logits = logits - np.tril(np.ones((sx, sx), np.float32), -1) * INF
        return logits

    def mcce(y_true, y_pred):
        bx, hx, sx, _ = y_pred.shape
        flat = y_true[..., 0].astype(np.int64) * sx + y_true[..., 1]
        yp = y_pred.reshape(bx, hx, sx * sx).astype(np.float64)
        total = 0.0
        for b in range(bx):
            for h in range(hx):
                f = flat[b, h]
                live = f != 0
                lv = yp[b, h][f]
                pos = np.log1p(np.sum(np.exp(-lv[live])))
                neg_terms = yp[b, h].copy()
                neg_terms[0] = -np.inf
                neg_terms[np.unique(f)] = -np.inf
                neg = np.log1p(np.sum(np.exp(neg_terms)))
                total += pos + neg
        return total

    loss = 0.0
    loss += mcce(gt_entity, gp(x, W_ent, b_ent, attention_mask, 2, True, True))
    loss += mcce(gt_head, gp(x, W_head, b_head, attention_mask, 1, False, False))
    loss += mcce(gt_tail, gp(x, W_tail, b_tail, attention_mask, 1, False, False))
    return np.array(loss, dtype=np.float32)


def _build_inputs(hidden_b, emb_rows):
    """Per-example xT8a [128,8192], xT8b [64,2048] fp8 from x_aug [1152,1024]."""
    X = np.zeros((KPAD, S), np.float32)
    X[:HID] = hidden_b.T
    X[HID:HID + LAB] = emb_rows.T
    X[HID + LAB] = 1.0
    X8 = X.astype(ml_dtypes.float8_e4m3)
    a = X8[:1024].reshape(4, 2, 128, S).transpose(2, 0, 1, 3).reshape(128, 8192)
    b = X8[1024:1152].reshape(2, 64, S).transpose(1, 0, 2).reshape(64, 2048)
    return np.ascontiguousarray(a), np.ascontiguousarray(b)


def _build_weights(W_all, b_all):
    """wtb8m [128, 6400] fp8: layout [p][t][kt][i][m] (alpha-scaled slots)."""
    slot = _slot_map()
    W8 = np.zeros((MSLOT, KPAD), np.float32)
    live = slot >= 0
    W8[live, :HID + LAB] = ALPHA * W_all[slot[live]]
    W8[live, HID + LAB] = ALPHA * b_all[slot[live]]
    W8 = W8.astype(ml_dtypes.float8_e4m3)
    # out[p, t, kt, i, m] = W8[t*128 + m, c(kt, i, p)]
    out = np.zeros((128, 5, 5, 2, 128), ml_dtypes.float8_e4m3)
    WT = W8.T                                            # [1152, 640]
    a = WT[:1024].reshape(4, 2, 128, 5, 128)             # [kt, i, p, t, m]
    out[:, :, :4] = a.transpose(2, 3, 0, 1, 4)
    b = WT[1024:1152].reshape(2, 64, 5, 128)             # [i, p, t, m]
    out[:64, :, 4] = b.transpose(1, 2, 0, 3)
    return np.ascontiguousarray(out.reshape(128, 6400))


def kernel(hidden, entity_labels, attention_mask, gt_entity, gt_head, gt_tail,
           ent_emb, W_ent, b_ent, W_head, b_head, W_tail, b_tail,
           _want_trace=False):
    hidden = np.asarray(hidden, np.float32)
    entity_labels = np.asarray(entity_labels)
    attention_mask = np.asarray(attention_mask, np.float32)
    ent_emb = np.asarray(ent_emb, np.float32)

    if not np.all(attention_mask == 1.0):
        return _reference_numpy(
            hidden, entity_labels, attention_mask, np.asarray(gt_entity),
            np.asarray(gt_head), np.asarray(gt_tail), ent_emb,
            np.asarray(W_ent, np.float32), np.asarray(b_ent, np.float32),
            np.asarray(W_head, np.float32), np.asarray(b_head, np.float32),
            np.asarray(W_tail, np.float32), np.asarray(b_tail, np.float32))

    W_all = np.concatenate(
        [np.asarray(W_ent, np.float32), np.asarray(W_head, np.float32),
         np.asarray(W_tail, np.float32)], axis=0)       # [544, 1088]
    b_all = np.concatenate(
        [np.asarray(b_ent, np.float32), np.asarray(b_head, np.float32),
         np.asarray(b_tail, np.float32)], axis=0)       # [544]

    wtb8m = _build_weights(W_all, b_all)
    cosT, sinT, auxm = _host_tables()

    in_maps = []
    for b in range(B):
        xa, xb = _build_inputs(hidden[b], ent_emb[entity_labels[b]])
        in_maps.append(dict(xT8a=xa, xT8b=xb, wtb8m=wtb8m,
                            cosT=cosT, sinT=sinT, aux=auxm))

    nc = _get_nc()
    res = run_bass_kernel_spmd(nc, in_maps, core_ids=list(range(NCORES)),
                               trace=_want_trace)

    # (gq, gk, tril?, sums col range, gt getter)
    heads = [
        (G_QHEAD, G_KHEAD, 0, 4, lambda b: np.asarray(gt_head)[b, 0]),
        (G_QTAIL, G_KTAIL, 4, 8, lambda b: np.asarray(gt_tail)[b, 0]),
        (G_QE0, G_KE0, 8, 11, lambda b: np.asarray(gt_entity)[b, 0]),
        (G_QE1, G_KE1, 11, 14, lambda b: np.asarray(gt_entity)[b, 1]),
    ]
    inv_a = 1.0 / ALPHA
    total = 0.0
    for b in range(B):
        out = res.results[b]
        sums_v = np.asarray(out["sums"], np.float64)       # [128, 14]
        qkv = np.asarray(out["qkout"], np.float64) * inv_a  # [8, 68, 1024]
        for (gq, gk, c0, c1, getgt) in heads:
            E = float(np.sum(sums_v[:, c0:c1]))
            total += _mcce_host(E, qkv[gq], qkv[gk], getgt(b))

    if _want_trace:
        kernel._last_results = res
    return np.array(total, dtype=np.float32)


# revision 19
# speedup vs baseline: 1.0091x; 1.0091x over previous
"""Bass/Trainium2 kernel for nn_GPREDecoder (GlobalPointer relation-extraction loss).

Strategy: data-parallel over batch (B=8 -> 8 cores, 1 example per core).
Per example the device computes:
  - fp8 DoubleRow projection  projT = (alpha*W_all) @ x_aug.T  (bias folded),
    output channels permuted so q_head/k_head/q_ent0/k_ent0/q_ent1 land at
    row 0 of the five 128-row psum m-tiles (direct SBUF views after a bf16
    evacuation cast); the 3 remaining groups are regrouped by SBUF->SBUF DMA.
  - RoPE on DVE: stream_shuffle pair-swap + sign-folded sin table
    (rot(q) = q*cos + swap(q)*sin'), no J matmul, all bf16 2x-mode ops.
  - per-head S x S logits in bf16 on PE, tril masks added by identity-matmul
    accumulation, exp(SCALE/alpha^2 * logit) on ACT with fused per-row
    accumulation into a [128, 14] sums tile (2048-wide spans).
  - bf16 q/k tensors DMA'd out for the host-side multilabel-CE corrections
    (gathers of the 64 ground-truth pairs, computed in float64).
"""

import ml_dtypes
import numpy as np
from contextlib import ExitStack

import concourse.bass as bass
import concourse.mybir as mybir
import concourse.tile as tile
from concourse import bacc
from concourse.bass_utils import run_bass_kernel_spmd

B, S, HID, LAB = 8, 1024, 1024, 64
HD = 68
SCALE = 1.0 / HD**0.5
INF = 1.0e12
NCORES = 8
ALPHA = 16.0                  # fp8 weight pre-scale; exp scale divides alpha^2
ACT_SCALE = SCALE / (ALPHA * ALPHA)
NEG_BIG = -1.0e9 * ALPHA * ALPHA  # additive mask units match scaled logits
KPAD = 1152                   # 4 full double-row k-tiles (256ch) + 1 half (128ch)
MSLOT = 640                   # 5 m-tiles x 128 permuted output-channel slots

# qkout group order (our choice):
G_QHEAD, G_KHEAD, G_QTAIL, G_KTAIL, G_QE0, G_KE0, G_QE1, G_KE1 = range(8)
# original row offset of each 68-row group in W_all = [W_ent; W_head; W_tail]
_ORIG = {G_QE0: 0, G_KE0: 68, G_QE1: 136, G_KE1: 204,
         G_QHEAD: 272, G_KHEAD: 340, G_QTAIL: 408, G_KTAIL: 476}
# row-0 groups of m-tiles 0..4 (direct views of the dense tiles)
_ROW0 = [G_QHEAD, G_KHEAD, G_QE0, G_KE0, G_QE1]
# spill groups: (src_tile, src_row, cnt, dst_row) pieces
_SPILL = {
    G_KE1: [(0, 68, 60, 0), (1, 68, 8, 60)],
    G_QTAIL: [(1, 76, 52, 0), (2, 68, 16, 52)],
    G_KTAIL: [(2, 84, 44, 0), (3, 68, 24, 44)],
}
# tril m-tile widths and ACT packs: lists of (m, local_col) per pack
_TRIL_W = [S - 128 * m for m in range(8)]
_TRIL_PACKS = [
    [(0, 0), (1, 1024)],            # span 1920
    [(2, 0), (3, 768), (4, 1408)],  # span 1920
    [(5, 0), (6, 384), (7, 640)],   # span 768
]
_TRIL_SPANS = [1920, 1920, 768]

# heads: (name, q_operand, k_operand, tril?) resolved at build time
# ACT instruction order: head x8 singles, tail x4 pairs, ent0 x3, ent1 x3
N_ACC = 18


def _slot_map():
    """slot (0..639) -> original W_all row, or -1 for pad."""
    slot = np.full(MSLOT, -1, np.int64)
    for t, g in enumerate(_ROW0):
        slot[t * 128: t * 128 + 68] = np.arange(_ORIG[g], _ORIG[g] + 68)
    for g, pieces in _SPILL.items():
        for (t, r, cnt, d) in pieces:
            slot[t * 128 + r: t * 128 + r + cnt] = np.arange(
                _ORIG[g] + d, _ORIG[g] + d + cnt)
    return slot


def _chunks_in_bank(base, lo, w):
    """Split [lo, lo+w) cols (absolute base+lo in the psum tile) at the
    512-col bank grid. Returns [(off, n)] with off relative to lo."""
    out = []
    off = 0
    while off < w:
        a = base + lo + off
        n = min(512 - (a % 512), w - off)
        out.append((off, n))
        off += n
    return out


def _build_nc():
    f32 = mybir.dt.float32
    bf16 = mybir.dt.bfloat16
    f8 = mybir.dt.float8e4
    Exp = mybir.ActivationFunctionType.Exp
    DR = mybir.MatmulPerfMode.DoubleRow

    nc = bacc.Bacc("TRN2", target_bir_lowering=False)

    xT8a = nc.dram_tensor("xT8a", [128, 8192], f8, kind="ExternalInput")
    xT8b = nc.dram_tensor("xT8b", [64, 2048], f8, kind="ExternalInput")
    # m-major weights: per (partition, m-tile) run of 5kt x 2 x 128 = 1280B
    wtb8m = nc.dram_tensor("wtb8m", [128, 6400], f8, kind="ExternalInput")
    cosT = nc.dram_tensor("cosT", [HD, S], bf16, kind="ExternalInput")
    sinT = nc.dram_tensor("sinT", [HD, S], bf16, kind="ExternalInput")
    aux = nc.dram_tensor("aux", [128, 256], bf16, kind="ExternalInput")
    sums = nc.dram_tensor("sums", [128, N_ACC], f32, kind="ExternalOutput")
    qkout = nc.dram_tensor("qkout", [8, HD, S], bf16, kind="ExternalOutput")

    with tile.TileContext(nc) as tc, ExitStack() as ctx:
        singles = ctx.enter_context(tc.tile_pool(name="singles", bufs=1))
        scratch = ctx.enter_context(tc.tile_pool(name="scratch", bufs=2))

        xT_sb = singles.tile([128, 5, 2, S], f8, tag="xT_sb", name="xT_sb")
        wtb_sb = singles.tile([128, 5, 5, 2, 128], f8, tag="wtb_sb",
                              name="wtb_sb")  # [p, t, kt, i, m]
        cos_sb = singles.tile([HD, S], bf16, tag="cos_sb", name="cos_sb")
        sin_sb = singles.tile([HD, S], bf16, tag="sin_sb", name="sin_sb")
        aux_sb = singles.tile([128, 256], bf16, tag="aux_sb", name="aux_sb")
        dense = [singles.tile([128, S], bf16, tag=f"dense{t}", name=f"dense{t}")
                 for t in range(5)]
        qk_sp = {g: singles.tile([HD, S], bf16, tag=f"sp{g}", name=f"sp{g}")
                 for g in _SPILL}
        qk_rope = {g: singles.tile([HD, S], bf16, tag=f"rp{g}", name=f"rp{g}")
                   for g in (G_QE0, G_KE0, G_QE1, G_KE1)}
        sums_sb = singles.tile([128, N_ACC], f32, tag="sums_sb", name="sums_sb")
        dummy = singles.tile([1, 8], f32, tag="dummy", name="dummy")

        I_sb = aux_sb[:, 0:128]
        mask_sb = aux_sb[:, 128:256]

        ps = ctx.enter_context(tc.tile_pool(name="ps", bufs=1, space="PSUM"))
        big = ps.tile([128, 4096], f32, tag="big", name="big")

        # ---- input DMAs: critical bytes (wtb t0/t1 + all xT) on the two HW
        # rings; wtb t2-t4 + constants on the gpsimd software DGE ----
        nc.vector.memset(dummy[:], 0.0)

        wtb8m_r = wtb8m.rearrange("p (t kt i m) -> p t kt i m", t=5, kt=5, i=2)
        xT8a_r = xT8a.rearrange("p (o i f) -> p o i f", o=4, i=2)

        nc.sync.dma_start(out=wtb_sb[:, 0], in_=wtb8m_r[:, 0])
        nc.sync.dma_start(out=xT_sb[:, 0:1], in_=xT8a_r[:, 0:1])
        nc.sync.dma_start(out=xT_sb[:, 1:2], in_=xT8a_r[:, 1:2])
        nc.sync.dma_start(out=xT_sb[0:64, 4], in_=xT8b.rearrange(
            "p (i f) -> p i f", i=2)[:, :, :])
        nc.scalar.dma_start(out=wtb_sb[:, 1], in_=wtb8m_r[:, 1])
        nc.scalar.dma_start(out=xT_sb[:, 2:4], in_=xT8a_r[:, 2:4])
        nc.scalar.activation(dummy[:], dummy[:], Exp)  # exp table warm
        nc.gpsimd.dma_start(out=wtb_sb[:, 2], in_=wtb8m_r[:, 2])
        nc.gpsimd.dma_start(out=wtb_sb[:, 3], in_=wtb8m_r[:, 3])
        nc.gpsimd.dma_start(out=sin_sb[:, :], in_=sinT[:, :])
        nc.gpsimd.dma_start(out=cos_sb[:, :], in_=cosT[:, :])
        nc.gpsimd.dma_start(out=wtb_sb[:, 4], in_=wtb8m_r[:, 4])
        nc.gpsimd.dma_start(out=aux_sb[:, :], in_=aux[:, :])

        # ---- helpers ----
        def proj(tiles_units, kt_lo=0, kt_hi=5):
            """Project m-tiles into psum units, kt-major (DMA-paced)."""
            for kt in range(kt_lo, kt_hi):
                p_hi = 64 if kt == 4 else 128
                for (t, u) in tiles_units:
                    base = u * 1024
                    for c in (0, 512):
                        nc.tensor.matmul(
                            big[0:128, base + c: base + c + 512],
                            wtb_sb[0:p_hi, t, kt, :, :],
                            xT_sb[0:p_hi, kt, :, c:c + 512],
                            start=(kt == 0), stop=(kt == 4),
                            perf_mode=DR,
                        )

        def evac(t, u, eng):
            src = big[0:128, u * 1024:(u + 1) * 1024]
            if eng == "act":
                nc.scalar.copy(out=dense[t][:, :], in_=src)
            else:
                nc.vector.tensor_copy(out=dense[t][:, :], in_=src)

        def nt_round(q_ap, k_ap, m, u):
            """One non-tril round m into psum unit u (1024 cols)."""
            base = u * 1024
            for c in (0, 512):
                nc.tensor.matmul(
                    big[0:128, base + c: base + c + 512],
                    q_ap[:, m * 128:(m + 1) * 128],
                    k_ap[:, c:c + 512],
                    start=True, stop=True,
                )

        def pair_round(q_ap, k_ap, r0, u0):
            """Two non-tril rounds r0, r0+1 into units u0, u0+1 (2048 span)."""
            for j in (0, 1):
                nt_round(q_ap, k_ap, r0 + j, u0 + j)

        def tril_pack(q_ap, k_ap, pack, u0):
            """One tril ACT pack into units u0,u0+1; mask + logits matmuls."""
            base = u0 * 1024
            for (m, lo) in pack:
                w = _TRIL_W[m]
                g0 = 128 * m
                # diag block: mask first (start), logits joins (stop)
                nc.tensor.matmul(
                    big[0:128, base + lo: base + lo + 128],
                    I_sb, mask_sb, start=True, stop=False)
                nc.tensor.matmul(
                    big[0:128, base + lo: base + lo + 128],
                    q_ap[:, g0:g0 + 128], k_ap[:, g0:g0 + 128],
                    start=False, stop=True)
                for (off, n) in _chunks_in_bank(base, lo + 128, w - 128):
                    nc.tensor.matmul(
                        big[0:128, base + lo + 128 + off:
                            base + lo + 128 + off + n],
                        q_ap[:, g0:g0 + 128],
                        k_ap[:, g0 + 128 + off: g0 + 128 + off + n],
                        start=True, stop=True)

        acc_i = [0]

        def act_span(u0, span):
            i = acc_i[0]
            acc_i[0] += 1
            ap = big[0:128, u0 * 1024: u0 * 1024 + span]
            nc.scalar.activation(ap, ap, Exp, scale=ACT_SCALE,
                                 accum_out=sums_sb[:, i:i + 1])

        def rope(g, src_ap):
            """qk_rope[g] = src*cos + pairswap(src)*sin' (DVE)."""
            sh = scratch.tile([HD, S], bf16, tag="sh", name=f"sh{g}")
            tmp = scratch.tile([HD, S], bf16, tag="tmp", name=f"tmp{g}")
            swap_mask = [i ^ 1 for i in range(32)]
            nc.vector.stream_shuffle(sh[:, :], src_ap, swap_mask)
            nc.vector.tensor_tensor(tmp[:, :], sh[:, :], sin_sb[:, :],
                                    mybir.AluOpType.mult)
            nc.vector.tensor_tensor(qk_rope[g][:, :], src_ap, cos_sb[:, :],
                                    mybir.AluOpType.mult)
            nc.vector.tensor_tensor(qk_rope[g][:, :], qk_rope[g][:, :],
                                    tmp[:, :], mybir.AluOpType.add)

        # ---- PE: proj01 fully, proj23 kt0-1 (fills PE while evacs run) ----
        proj([(0, 0), (1, 1)])
        proj([(2, 2), (3, 3)], 0, 2)

        # ---- evacuations: dense0 on DVE, dense1 on ACT ----
        evac(0, 0, "dve")
        evac(1, 1, "act")

        q_head = dense[0][0:HD, :]
        k_head = dense[1][0:HD, :]

        # ---- head: 8 single rounds double-buffered on U0/U1 only (U2/U3
        # still hold proj23); proj23 kt2-4 squeezed in after round 1 ----
        nt_round(q_head, k_head, 0, 0)
        act_span(0, 1024)
        nt_round(q_head, k_head, 1, 1)
        act_span(1, 1024)
        proj([(2, 2), (3, 3)], 2, 5)
        for m in range(2, 8):
            nt_round(q_head, k_head, m, m % 2)
            act_span(m % 2, 1024)

        evac(2, 2, "dve")
        evac(3, 3, "dve")
        q_ent0_raw = dense[2][0:HD, :]
        k_ent0_raw = dense[3][0:HD, :]

        # ---- spills + qkout on the sync HW ring (idle after inputs);
        # tail pieces first (needed sooner), k_ent1 later (rope is slower) --
        for g in (G_QTAIL, G_KTAIL, G_KE1):
            for (t, r, cnt, d) in _SPILL[g]:
                nc.sync.dma_start(out=qk_sp[g][d:d + cnt, :],
                                  in_=dense[t][r:r + cnt, :])
        nc.sync.dma_start(out=qkout[G_QHEAD], in_=q_head)
        nc.sync.dma_start(out=qkout[G_KHEAD], in_=k_head)
        for g in (G_QTAIL, G_KTAIL):
            nc.sync.dma_start(out=qkout[g], in_=qk_sp[g][:, :])

        # ---- DVE: rope for ent0 (ent1 after evac4) ----
        rope(G_QE0, q_ent0_raw)
        rope(G_KE0, k_ent0_raw)

        q_tail = qk_sp[G_QTAIL][:, :]
        k_tail = qk_sp[G_KTAIL][:, :]
        pair_round(q_tail, k_tail, 0, 2)      # tailA @U23 (after evac23)
        act_span(2, 2048)
        proj([(4, 0)])                        # proj4 @U0 after head drained
        evac(4, 0, "dve")
        rope(G_QE1, dense[4][0:HD, :])
        rope(G_KE1, qk_sp[G_KE1][:, :])
        for g in (G_QE0, G_KE0, G_QE1, G_KE1):
            nc.sync.dma_start(out=qkout[g], in_=qk_rope[g][:, :])

        pair_round(q_tail, k_tail, 2, 2)      # tailB @U23 (single-buffer)
        act_span(2, 2048)
        pair_round(q_tail, k_tail, 4, 0)      # tailC @U01 (after evac4)
        act_span(0, 2048)
        pair_round(q_tail, k_tail, 6, 2)      # tailD @U23
        act_span(2, 2048)

        qe0, ke0 = qk_rope[G_QE0][:, :], qk_rope[G_KE0][:, :]
        qe1, ke1 = qk_rope[G_QE1][:, :], qk_rope[G_KE1][:, :]
        units = [0, 2, 0, 2, 0, 2]
        plan = [(qe0, ke0, 0), (qe0, ke0, 1), (qe0, ke0, 2),
                (qe1, ke1, 0), (qe1, ke1, 1), (qe1, ke1, 2)]
        for (qa, ka, pi), u0 in zip(plan, units):
            tril_pack(qa, ka, _TRIL_PACKS[pi], u0)
            act_span(u0, _TRIL_SPANS[pi])

        # sums out on the scalar queue: no cross-engine hop after last exp
        nc.scalar.dma_start(out=sums[:, :], in_=sums_sb[:, :])

    nc.finalize()
    return nc


_NC_CACHE = None


def _get_nc():
    global _NC_CACHE
    if _NC_CACHE is None:
        _NC_CACHE = _build_nc()
    return _NC_CACHE


def _host_tables():
    pos = np.arange(S, dtype=np.float64)[:, None]
    inv = np.power(10000.0, -2.0 * np.arange(HD // 2, dtype=np.float64) / HD)
    ang = pos * inv                                    # [S, 34]
    cosr = np.repeat(np.cos(ang), 2, axis=1).T         # [68, S]
    sinr = np.repeat(np.sin(ang), 2, axis=1).T
    # sign-folded sin: rot = x*cos + swap(x)*sin'; sin'[2i] = -sin, [2i+1] = +
    sgn = np.where(np.arange(HD) % 2 == 0, -1.0, 1.0)[:, None]
    cosT = cosr.astype(ml_dtypes.bfloat16)
    sinT = (sinr * sgn).astype(ml_dtypes.bfloat16)
    auxm = np.zeros((128, 256), np.float32)
    auxm[:, 0:128] = np.eye(128, dtype=np.float32)
    auxm[:, 128:256] = np.where(
        np.arange(128)[None, :] >= np.arange(128)[:, None], 0.0, NEG_BIG)
    return cosT, sinT, auxm.astype(ml_dtypes.bfloat16)


def _mcce_host(E_dev, q, k, gt):
    """pos/neg multilabel-CE for one (example, head). q,k: [68,S] f64; gt [P,2]."""
    i = gt[:, 0].astype(np.int64)
    j = gt[:, 1].astype(np.int64)
    flat = i * S + j
    lv = np.sum(q[:, i] * k[:, j], axis=0) * SCALE     # [P]
    live = flat != 0
    pos_loss = np.log1p(np.sum(np.exp(-lv[live])))
    l00 = float(np.sum(q[:, 0] * k[:, 0]) * SCALE)
    uf, ui = np.unique(flat, return_index=True)
    keep = uf != 0
    excl = np.exp(l00) + np.sum(np.exp(lv[ui[keep]]))
    neg_loss = np.log1p(E_dev - excl)
    return pos_loss + neg_loss


def _reference_numpy(hidden, entity_labels, attention_mask, gt_entity, gt_head,
                     gt_tail, ent_emb, W_ent, b_ent, W_head, b_head, W_tail,
                     b_tail):
    """Slow exact numpy fallback (used only if attention_mask is not all-ones)."""
    x = np.concatenate([hidden, ent_emb[entity_labels]], axis=-1)

    def rope_np(v):
        b, s, h, d = v.shape
        pos = np.arange(s, dtype=np.float32)[:, None]
        inv = np.power(10000.0, -2.0 * np.arange(d // 2, dtype=np.float32) / d)
        ang = pos * inv
        sin = np.repeat(np.sin(ang), 2, axis=-1)[None, :, None, :]
        cos = np.repeat(np.cos(ang), 2, axis=-1)[None, :, None, :]
        v2 = np.stack([-v[..., 1::2], v[..., ::2]], axis=-1).reshape(v.shape)
        return v * cos + v2 * sin

    def gp(x, W, b, mask, heads, use_rope, tril):
        bx, sx, _ = x.shape
        proj = (x @ W.T + b).reshape(bx, sx, heads, 2 * HD)
        qw, kw = proj[..., :HD], proj[..., HD:]
        if use_rope:
            qw, kw = rope_np(qw), rope_np(kw)
        logits = np.einsum('bmhd,bnhd->bhmn', qw, kw) * SCALE
        pad = mask[:, None, None, :]
        logits = logits * pad - (1.0 - pad) * INF
        if tril:
            logits = logits - np.tril(np.ones((sx, sx), np.float32), -1) * INF
        return logits

    def mcce(y_true, y_pred):
        bx, hx, sx, _ = y_pred.shape
        flat = y_true[..., 0].astype(np.int64) * sx + y_true[..., 1]
        yp = y_pred.reshape(bx, hx, sx * sx).astype(np.float64)
        total = 0.0
        for b in range(bx):
            for h in range(hx):
                f = flat[b, h]
                live = f != 0
                lv = yp[b, h][f]
                pos = np.log1p(np.sum(np.exp(-lv[live])))
                neg_terms = yp[b, h].copy()
                neg_terms[0] = -np.inf
                neg_terms[np.unique(f)] = -np.inf
                neg = np.log1p(np.sum(np.exp(neg_terms)))
                total += pos + neg
        return total

    loss = 0.0
    loss += mcce(gt_entity, gp(x, W_ent, b_ent, attention_mask, 2, True, True))
    loss += mcce(gt_head, gp(x, W_head, b_head, attention_mask, 1, False, False))
    loss += mcce(gt_tail, gp(x, W_tail, b_tail, attention_mask, 1, False, False))
    return np.array(loss, dtype=np.float32)


def _build_inputs(hidden_b, emb_rows):
    """Per-example xT8a [128,8192], xT8b [64,2048] fp8 from x_aug [1152,1024]."""
    X = np.zeros((KPAD, S), np.float32)
    X[:HID] = hidden_b.T
    X[HID:HID + LAB] = emb_rows.T
    X[HID + LAB] = 1.0
    X8 = X.astype(ml_dtypes.float8_e4m3)
    a = X8[:1024].reshape(4, 2, 128, S).transpose(2, 0, 1, 3).reshape(128, 8192)
    b = X8[1024:1152].reshape(2, 64, S).transpose(1, 0, 2).reshape(64, 2048)
    return np.ascontiguousarray(a), np.ascontiguousarray(b)


def _build_weights(W_all, b_all):
    """wtb8m [128, 6400] fp8: layout [p][t][kt][i][m] (alpha-scaled slots)."""
    slot = _slot_map()
    W8 = np.zeros((MSLOT, KPAD), np.float32)
    live = slot >= 0
    W8[live, :HID + LAB] = ALPHA * W_all[slot[live]]
    W8[live, HID + LAB] = ALPHA * b_all[slot[live]]
    W8 = W8.astype(ml_dtypes.float8_e4m3)
    # out[p, t, kt, i, m] = W8[t*128 + m, c(kt, i, p)]
    out = np.zeros((128, 5, 5, 2, 128), ml_dtypes.float8_e4m3)
    WT = W8.T                                            # [1152, 640]
    a = WT[:1024].reshape(4, 2, 128, 5, 128)             # [kt, i, p, t, m]
    out[:, :, :4] = a.transpose(2, 3, 0, 1, 4)
    b = WT[1024:1152].reshape(2, 64, 5, 128)             # [i, p, t, m]
    out[:64, :, 4] = b.transpose(1, 2, 0, 3)
    return np.ascontiguousarray(out.reshape(128, 6400))


def kernel(hidden, entity_labels, attention_mask, gt_entity, gt_head, gt_tail,
           ent_emb, W_ent, b_ent, W_head, b_head, W_tail, b_tail,
           _want_trace=False):
    hidden = np.asarray(hidden, np.float32)
    entity_labels = np.asarray(entity_labels)
    attention_mask = np.asarray(attention_mask, np.float32)
    ent_emb = np.asarray(ent_emb, np.float32)

    if not np.all(attention_mask == 1.0):
        return _reference_numpy(
            hidden, entity_labels, attention_mask, np.asarray(gt_entity),
            np.asarray(gt_head), np.asarray(gt_tail), ent_emb,
            np.asarray(W_ent, np.float32), np.asarray(b_ent, np.float32),
            np.asarray(W_head, np.float32), np.asarray(b_head, np.float32),
            np.asarray(W_tail, np.float32), np.asarray(b_tail, np.float32))

    W_all = np.concatenate(
        [np.asarray(W_ent, np.float32), np.asarray(W_head, np.float32),
         np.asarray(W_tail, np.float32)], axis=0)       # [544, 1088]
    b_all = np.concatenate(
        [np.asarray(b_ent, np.float32), np.asarray(b_head, np.float32),
         np.asarray(b_tail, np.float32)], axis=0)       # [544]

    wtb8m = _build_weights(W_all, b_all)
    cosT, sinT, auxm = _host_tables()

    in_maps = []
    for b in range(B):
        xa, xb = _build_inputs(hidden[b], ent_emb[entity_labels[b]])
        in_maps.append(dict(xT8a=xa, xT8b=xb, wtb8m=wtb8m,
                            cosT=cosT, sinT=sinT, aux=auxm))

    nc = _get_nc()
    res = run_bass_kernel_spmd(nc, in_maps, core_ids=list(range(NCORES)),
                               trace=_want_trace)

    # (gq, gk, tril?, sums col range, gt getter)
    heads = [
        (G_QHEAD, G_KHEAD, 0, 8, lambda b: np.asarray(gt_head)[b, 0]),
        (G_QTAIL, G_KTAIL, 8, 12, lambda b: np.asarray(gt_tail)[b, 0]),
        (G_QE0, G_KE0, 12, 15, lambda b: np.asarray(gt_entity)[b, 0]),
        (G_QE1, G_KE1, 15, 18, lambda b: np.asarray(gt_entity)[b, 1]),
    ]
    inv_a = 1.0 / ALPHA
    total = 0.0
    for b in range(B):
        out = res.results[b]
        sums_v = np.asarray(out["sums"], np.float64)       # [128, 14]
        qkv = np.asarray(out["qkout"], np.float64) * inv_a  # [8, 68, 1024]
        for (gq, gk, c0, c1, getgt) in heads:
            E = float(np.sum(sums_v[:, c0:c1]))
            total += _mcce_host(E, qkv[gq], qkv[gk], getgt(b))

    if _want_trace:
        kernel._last_results = res
    return np.array(total, dtype=np.float32)
